# revision 9
# baseline (speedup 1.0000x reference)
"""ArcticMoE (dense top-2 MoE, 8 experts) — Trainium2 Bass kernel, 8 NeuronCores.

Sharding: expert-parallel. Core c receives expert c's weights (w1/w3/w2[c]),
the full token stream, and gate_w with columns rolled so core c's own expert
sits at column 0 (top-k selection and the aux loss are permutation-invariant).
Each core computes coef_c[t] * (silu(x@w1c) * (x@w3c)) @ w2c for all tokens;
ReduceScatter(add) sums over experts and scatters token shards; the host
concatenates the per-core shards.

Tokens are processed in two halves so the first half's ReduceScatter overlaps
the second half's compute. The host permutes token rows such that an 8-way
128-row scatter of each half lands exactly on sub-blocks of each core's final
256-row shard — no inverse permutation is needed on the output.

Compute dtype: float32r (full-rate PE streaming with fp32 storage) for the
big matmuls; the router matmuls run in plain fp32 so the top-2 selection
matches the fp32 reference. The aux load-balancing loss is computed on-device
via ones-vector matmul column sums of the top-1/top-2 masks and softmax probs.
"""

import threading

import numpy as np

B, S, H, F, E = 2, 1024, 1024, 4096, 8
T = B * S                    # 2048 tokens
P = 128                      # partitions
NCORES = 8
TSH = T // NCORES            # 256-row output shard per core

nH = H // P                  # 8  k-tiles over hidden dim
nT = T // P                  # 16 token tiles
nF = F // P                  # 32 f tiles
NHALF = 2                    # token halves (RS overlap granularity)
TH = T // NHALF              # 1024 tokens per half
nTh = TH // P                # 8 token tiles per half
FCH = 512                    # f-chunk width (phase-1 -> phase-2 granularity)
fpc = FCH // P               # 4  f-tiles per chunk
nFC = F // FCH               # 8 chunks
TB = 512                     # token block (matmul moving dim)
nTBh = TH // TB              # 2 per half
HH = 512                     # h_out half width (phase-2 psum free dim)
nHH = H // HH                # 2

_lock = threading.Lock()
_compiled = {}


def _token_perm():
    # perm[h*TH + c*P + i] = c*TSH + h*P + i
    perm = np.empty(T, dtype=np.int64)
    idx = 0
    for h in range(NHALF):
        for c in range(NCORES):
            base = c * TSH + h * P
            perm[idx : idx + P] = np.arange(base, base + P)
            idx += P
    return perm


def _build():
    import concourse.mybir as mybir
    import concourse.tile as tile
    from concourse import bacc

    f32 = mybir.dt.float32
    f32r = mybir.dt.float32r
    AX = mybir.AxisListType.X
    AF = mybir.ActivationFunctionType
    OP = mybir.AluOpType

    nc = bacc.Bacc(
        "TRN2",
        target_bir_lowering=False,
        debug=False,
        enable_asserts=False,
        num_devices=NCORES,
    )

    x_ext = nc.dram_tensor("x", [H, T], f32, kind="ExternalInput").ap()
    gate_ext = nc.dram_tensor("gate", [H, E], f32, kind="ExternalInput").ap()
    w13_ext = nc.dram_tensor("w13", [nF, nH, P, 2 * P], f32, kind="ExternalInput").ap()
    w2_ext = nc.dram_tensor("w2", [nF, nHH, P, HH], f32, kind="ExternalInput").ap()
    out_ext = nc.dram_tensor("out", [TSH, H], f32, kind="ExternalOutput").ap()
    aux_ext = nc.dram_tensor("aux", [1, 1], f32, kind="ExternalOutput").ap()

    with tile.TileContext(nc) as tc:
        with (
            tc.tile_pool(name="const", bufs=1) as const_pool,
            tc.tile_pool(name="xt", bufs=1) as xt_pool,
            tc.tile_pool(name="acc", bufs=1) as acc_pool,
            tc.tile_pool(name="ht", bufs=1) as ht_pool,
            tc.tile_pool(name="wstream", bufs=2) as wstream,
            tc.tile_pool(name="silu", bufs=3) as silu_pool,
            tc.tile_pool(name="router", bufs=1) as router_pool,
            tc.tile_pool(name="rtmp", bufs=2) as rtmp_pool,
            tc.tile_pool(name="dram", bufs=1, space="DRAM") as dram_pool,
        ):
            ones = const_pool.tile([P, 1], f32)
            nc.vector.memset(ones, 1.0)
            gate_sb = const_pool.tile([P, nH * E], f32)
            for k in range(nH):
                nc.sync.dma_start(
                    gate_sb[:, k * E : (k + 1) * E],
                    gate_ext[k * P : (k + 1) * P, :],
                )

            xt = [xt_pool.tile([P, T], f32r, tag=f"xt{k}", name=f"xt{k}") for k in range(nH)]

            mask1 = router_pool.tile([P, nT * E], f32)
            mask2 = router_pool.tile([P, nT * E], f32)
            smn = router_pool.tile([P, nT * E], f32)
            coef = router_pool.tile([P, nT], f32)

            # ---- load xT directly (host supplies [H, T] layout) ----
            for tc4 in range(4):
                cs = slice(tc4 * (T // 4), (tc4 + 1) * (T // 4))
                for k in range(nH):
                    nc.sync.dma_start(
                        xt[k][:, cs], x_ext[k * P : (k + 1) * P, cs].bitcast(f32r)
                    )

            # ---- router: logits -> top-2 coefs + softmax stats (fp32 exact) ----
            with tc.tile_pool(name="lg_ps", bufs=2, space="PSUM") as lg_ps:
                for t in range(nT):
                    te = slice(t * E, (t + 1) * E)
                    ps = lg_ps.tile([P, E], f32, tag="lg", name="lg")
                    for k in range(nH):
                        nc.tensor.matmul(
                            ps,
                            xt[k][:, t * P : (t + 1) * P].bitcast(f32),
                            gate_sb[:, k * E : (k + 1) * E],
                            start=(k == 0),
                            stop=(k == nH - 1),
                        )
                    lt = rtmp_pool.tile([P, E], f32, tag="lt", name="lt")
                    nc.scalar.copy(lt, ps)
                    m1 = rtmp_pool.tile([P, 1], f32, tag="m1", name="m1")
                    nc.vector.reduce_max(m1, lt, axis=AX)
                    nm1 = rtmp_pool.tile([P, 1], f32, tag="nm1", name="nm1")
                    nc.vector.reduce_max(nm1, lt, axis=AX, negate=True)
                    nc.vector.tensor_scalar(mask1[:, te], lt, m1, None, op0=OP.is_ge)
                    lm = rtmp_pool.tile([P, E], f32, tag="lm", name="lm")
                    nc.vector.scalar_tensor_tensor(
                        lm, mask1[:, te], -1e30, lt, op0=OP.mult, op1=OP.add
                    )
                    m2 = rtmp_pool.tile([P, 1], f32, tag="m2", name="m2")
                    nc.vector.reduce_max(m2, lm, axis=AX)
                    nc.vector.tensor_scalar(mask2[:, te], lm, m2, None, op0=OP.is_ge)
                    d = rtmp_pool.tile([P, 1], f32, tag="d", name="d")
                    nc.vector.tensor_sub(d, m1, m2)
                    p1 = rtmp_pool.tile([P, 1], f32, tag="p1", name="p1")
                    nc.scalar.activation(p1, d, AF.Sigmoid)
                    p2 = rtmp_pool.tile([P, 1], f32, tag="p2", name="p2")
                    nc.scalar.activation(p2, d, AF.Sigmoid, scale=-1.0)
                    ca = rtmp_pool.tile([P, 1], f32, tag="ca", name="ca")
                    nc.vector.tensor_mul(ca, mask1[:, t * E : t * E + 1], p1)
                    cb = rtmp_pool.tile([P, 1], f32, tag="cb", name="cb")
                    nc.vector.tensor_mul(cb, mask2[:, t * E : t * E + 1], p2)
                    nc.vector.tensor_add(coef[:, t : t + 1], ca, cb)
                    # softmax over all 8 logits (aux loss)
                    sm = rtmp_pool.tile([P, E], f32, tag="sm", name="sm")
                    nc.scalar.activation(sm, lt, AF.Exp, bias=nm1)
                    ssum = rtmp_pool.tile([P, 1], f32, tag="ssum", name="ssum")
                    nc.vector.reduce_sum(ssum, sm, axis=AX)
                    rcp = rtmp_pool.tile([P, 1], f32, tag="rcp", name="rcp")
                    nc.vector.reciprocal(rcp, ssum)
                    nc.vector.tensor_scalar(smn[:, te], sm, rcp, None, op0=OP.mult)

            # ---- main expert FFN, one pass per token half; RS overlaps ----
            with (
                tc.tile_pool(name="ph1_ps", bufs=2, space="PSUM") as ph1_ps,
                tc.tile_pool(name="ph2_ps", bufs=3, space="PSUM") as ph2_ps,
            ):
                for hf in range(NHALF):
                    toff = hf * TH
                    acc = [
                        acc_pool.tile([P, H], f32, tag=f"acc{t}", name=f"acc{t}")
                        for t in range(nTh)
                    ]
                    for c in range(nFC):
                        ht = [
                            ht_pool.tile([P, TH], f32r, tag=f"ht{j}", name=f"ht{j}")
                            for j in range(fpc)
                        ]
                        # phase 1: hT[f, t] = silu(w1.T x) * (w3.T x)
                        for j in range(fpc):
                            ft = c * fpc + j
                            w1t, w3t = [], []
                            for k in range(nH):
                                a = wstream.tile([P, 2 * P], f32r, tag=f"w13_{k}", name=f"w13_{k}", bufs=3)
                                nc.sync.dma_start(a, w13_ext[ft, k].bitcast(f32r))
                                w1t.append(a[:, 0:P])
                                w3t.append(a[:, P : 2 * P])
                            for tb in range(nTBh):
                                xs = slice(toff + tb * TB, toff + (tb + 1) * TB)
                                ts_ = slice(tb * TB, (tb + 1) * TB)
                                g1 = ph1_ps.tile([P, TB], f32, tag="g1", name="g1")
                                g3 = ph1_ps.tile([P, TB], f32, tag="g3", name="g3")
                                for k in range(nH):
                                    nc.tensor.matmul(
                                        g1, w1t[k], xt[k][:, xs],
                                        start=(k == 0), stop=(k == nH - 1),
                                    )
                                for k in range(nH):
                                    nc.tensor.matmul(
                                        g3, w3t[k], xt[k][:, xs],
                                        start=(k == 0), stop=(k == nH - 1),
                                    )
                                s1 = silu_pool.tile([P, TB], f32, tag="s1", name="s1")
                                nc.scalar.activation(s1, g1, AF.Silu)
                                nc.vector.tensor_mul(ht[j][:, ts_], s1, g3)
                        # phase 2: acc[t, h] += hT.T @ w2
                        for hh in range(nHH):
                            hs = slice(hh * HH, (hh + 1) * HH)
                            w2t = []
                            for j in range(fpc):
                                ft = c * fpc + j
                                a = wstream.tile([P, HH], f32r, tag=f"w2_{j}", name=f"w2_{j}", bufs=3)
                                nc.sync.dma_start(a, w2_ext[ft, hh].bitcast(f32r))
                                w2t.append(a)
                            for t in range(nTh):
                                ps = ph2_ps.tile([P, HH], f32, tag="o", name="o")
                                for j in range(fpc):
                                    nc.tensor.matmul(
                                        ps, ht[j][:, t * P : (t + 1) * P], w2t[j],
                                        start=(j == 0), stop=(j == fpc - 1),
                                    )
                                if c == 0:
                                    nc.vector.tensor_copy(acc[t][:, hs], ps)
                                else:
                                    nc.vector.tensor_add(acc[t][:, hs], acc[t][:, hs], ps)

                    # scale by routing coef, write out, reduce-scatter this half
                    y_dram = dram_pool.tile([TH, H], f32, tag=f"y{hf}", name=f"y{hf}")
                    rs_dram = dram_pool.tile([P, H], f32, tag=f"rs{hf}", name=f"rs{hf}")
                    for t in range(nTh):
                        tg = hf * nTh + t
                        nc.scalar.activation(
                            acc[t], acc[t], AF.Copy, scale=coef[:, tg : tg + 1]
                        )
                        nc.scalar.dma_start(y_dram[t * P : (t + 1) * P, :], acc[t])
                    nc.gpsimd.collective_compute(
                        "ReduceScatter",
                        OP.add,
                        replica_groups=[list(range(NCORES))],
                        ins=[y_dram.opt()],
                        outs=[rs_dram.opt()],
                    )
                    nc.gpsimd.dma_start(out_ext[hf * P : (hf + 1) * P, :], rs_dram)

            # ---- aux loss: E/T^2 * sum_e (sum_t m1 + sum_t m2)_e * (sum_t sm)_e ----
            with tc.tile_pool(name="aux_ps", bufs=1, space="PSUM") as aux_ps:
                aps = aux_ps.tile([1, 3 * E], f32)
                for i, src in enumerate((mask1, mask2, smn)):
                    for t in range(nT):
                        nc.tensor.matmul(
                            aps[:, i * E : (i + 1) * E],
                            ones,
                            src[:, t * E : (t + 1) * E],
                            start=(t == 0),
                            stop=(t == nT - 1),
                        )
                asb = rtmp_pool.tile([1, 3 * E], f32, tag="asb", name="asb")
                nc.scalar.copy(asb, aps)
                a1 = rtmp_pool.tile([1, E], f32, tag="a1", name="a1")
                nc.vector.tensor_add(a1, asb[:, 0:E], asb[:, E : 2 * E])
                a2 = rtmp_pool.tile([1, E], f32, tag="a2", name="a2")
                nc.vector.tensor_mul(a2, a1, asb[:, 2 * E : 3 * E])
                a3 = rtmp_pool.tile([1, 1], f32, tag="a3", name="a3")
                nc.vector.reduce_sum(a3, a2, axis=AX)
                aux_sb = rtmp_pool.tile([1, 1], f32, tag="aux_sb", name="aux_sb")
                nc.scalar.mul(aux_sb, a3, float(E) / float(T * T))
                nc.gpsimd.dma_start(aux_ext, aux_sb)

    nc.compile()
    return nc


def _get_nc():
    with _lock:
        if "nc" not in _compiled:
            _compiled["nc"] = _build()
        return _compiled["nc"]


# test.py can set TRACE=True to capture a neuron profile; the resulting
# BassKernelResults lands in LAST_RESULT.
TRACE = False
LAST_RESULT = None


def kernel(**inputs):
    global LAST_RESULT
    from concourse.bass_utils import run_bass_kernel_spmd

    hs = np.ascontiguousarray(
        np.asarray(inputs["hidden_states"], dtype=np.float32).reshape(T, H)
    )
    gate_w = np.asarray(inputs["gate_w"], dtype=np.float32)
    w1 = np.asarray(inputs["w1"], dtype=np.float32)
    w3 = np.asarray(inputs["w3"], dtype=np.float32)
    w2 = np.asarray(inputs["w2"], dtype=np.float32)

    x_ht = np.ascontiguousarray(hs[_token_perm()].T)  # [H, T]
    # weight tiles repacked contiguous: w1+w3 packed [nF, nH, P, 2P], w2 [nF, nHH, P, HH]
    w1_t = w1.reshape(NCORES, nH, P, nF, P).transpose(0, 3, 1, 2, 4)
    w3_t = w3.reshape(NCORES, nH, P, nF, P).transpose(0, 3, 1, 2, 4)
    w13_t = np.concatenate([w1_t, w3_t], axis=-1)  # [NCORES, nF, nH, P, 2P]
    w2_t = w2.reshape(NCORES, nF, P, nHH, HH).transpose(0, 1, 3, 2, 4)

    nc = _get_nc()
    in_maps = []
    for c in range(NCORES):
        in_maps.append(
            {
                "x": x_ht,
                "gate": np.ascontiguousarray(np.roll(gate_w, -c, axis=1)),
                "w13": np.ascontiguousarray(w13_t[c]),
                "w2": np.ascontiguousarray(w2_t[c]),
            }
        )

    res = run_bass_kernel_spmd(nc, in_maps, core_ids=list(range(NCORES)), trace=TRACE)
    LAST_RESULT = res

    final = np.concatenate(
        [res.results[c]["out"] for c in range(NCORES)], axis=0
    ).reshape(B, S, H)
    aux = np.asarray(res.results[0]["aux"], dtype=np.float32).reshape(())
    return final, aux


# revision 10
# speedup vs baseline: 1.0797x; 1.0797x over previous
"""ArcticMoE (dense top-2 MoE, 8 experts) — Trainium2 Bass kernel, 8 NeuronCores.

Sharding: expert-parallel. Core c receives expert c's weights (w1/w3/w2[c]),
the full token stream, and gate_w with columns rolled so core c's own expert
sits at column 0 (top-k selection and the aux loss are permutation-invariant).
Each core computes coef_c[t] * (silu(x@w1c) * (x@w3c)) @ w2c for all tokens;
ReduceScatter(add) sums over experts and scatters token shards; the host
concatenates the per-core shards.

Tokens are processed in two halves so the first half's ReduceScatter overlaps
the second half's compute. The host permutes token rows such that an 8-way
128-row scatter of each half lands exactly on sub-blocks of each core's final
256-row shard — no inverse permutation is needed on the output.

Compute dtype: float32r (full-rate PE streaming with fp32 storage) for the
big matmuls; the router matmuls run in plain fp32 so the top-2 selection
matches the fp32 reference. The aux load-balancing loss is computed on-device
via ones-vector matmul column sums of the top-1/top-2 masks and softmax probs.
"""

import threading

import numpy as np

B, S, H, F, E = 2, 1024, 1024, 4096, 8
T = B * S                    # 2048 tokens
P = 128                      # partitions
NCORES = 8
TSH = T // NCORES            # 256-row output shard per core

nH = H // P                  # 8  k-tiles over hidden dim
nT = T // P                  # 16 token tiles
nF = F // P                  # 32 f tiles
NHALF = 2                    # token halves (RS overlap granularity)
TH = T // NHALF              # 1024 tokens per half
nTh = TH // P                # 8 token tiles per half
FCH = 512                    # f-chunk width (phase-1 -> phase-2 granularity)
fpc = FCH // P               # 4  f-tiles per chunk
nFC = F // FCH               # 8 chunks
TB = 512                     # token block (matmul moving dim)
nTBh = TH // TB              # 2 per half
HH = 512                     # h_out half width (phase-2 psum free dim)
nHH = H // HH                # 2

_lock = threading.Lock()
_compiled = {}


def _token_perm():
    # perm[h*TH + c*P + i] = c*TSH + h*P + i
    perm = np.empty(T, dtype=np.int64)
    idx = 0
    for h in range(NHALF):
        for c in range(NCORES):
            base = c * TSH + h * P
            perm[idx : idx + P] = np.arange(base, base + P)
            idx += P
    return perm


def _build():
    import concourse.mybir as mybir
    import concourse.tile as tile
    from concourse import bacc

    f32 = mybir.dt.float32
    f32r = mybir.dt.float32r
    AX = mybir.AxisListType.X
    AF = mybir.ActivationFunctionType
    OP = mybir.AluOpType

    nc = bacc.Bacc(
        "TRN2",
        target_bir_lowering=False,
        debug=False,
        enable_asserts=False,
        num_devices=NCORES,
    )

    x_ext = nc.dram_tensor("x", [H, T], f32, kind="ExternalInput").ap()
    gate_ext = nc.dram_tensor("gate", [H, E], f32, kind="ExternalInput").ap()
    w13_ext = nc.dram_tensor("w13", [nF, nH, P, 2 * P], f32, kind="ExternalInput").ap()
    w2_ext = nc.dram_tensor("w2", [nF, nHH, P, HH], f32, kind="ExternalInput").ap()
    out_ext = nc.dram_tensor("out", [TSH, H], f32, kind="ExternalOutput").ap()
    aux_ext = nc.dram_tensor("aux", [1, 1], f32, kind="ExternalOutput").ap()

    with tile.TileContext(nc) as tc:
        with (
            tc.tile_pool(name="const", bufs=1) as const_pool,
            tc.tile_pool(name="xt", bufs=1) as xt_pool,
            tc.tile_pool(name="acc", bufs=1) as acc_pool,
            tc.tile_pool(name="ht", bufs=1) as ht_pool,
            tc.tile_pool(name="wstream", bufs=2) as wstream,
            tc.tile_pool(name="silu", bufs=3) as silu_pool,
            tc.tile_pool(name="router", bufs=1) as router_pool,
            tc.tile_pool(name="rtmp", bufs=2) as rtmp_pool,
            tc.tile_pool(name="dram", bufs=1, space="DRAM") as dram_pool,
        ):
            ones = const_pool.tile([P, 1], f32)
            nc.vector.memset(ones, 1.0)
            gate_sb = const_pool.tile([P, nH * E], f32)
            for k in range(nH):
                nc.sync.dma_start(
                    gate_sb[:, k * E : (k + 1) * E],
                    gate_ext[k * P : (k + 1) * P, :],
                )

            xt = [xt_pool.tile([P, T], f32r, tag=f"xt{k}", name=f"xt{k}") for k in range(nH)]

            mask1 = router_pool.tile([P, nT * E], f32)
            mask2 = router_pool.tile([P, nT * E], f32)
            smn = router_pool.tile([P, nT * E], f32)
            coef = router_pool.tile([P, nT], f32)

            # ---- load xT directly (host supplies [H, T] layout) ----
            for tc4 in range(4):
                cs = slice(tc4 * (T // 4), (tc4 + 1) * (T // 4))
                for k in range(nH):
                    nc.sync.dma_start(
                        xt[k][:, cs], x_ext[k * P : (k + 1) * P, cs].bitcast(f32r)
                    )

            # ---- router: logits -> top-2 coefs + softmax stats (fp32 exact) ----
            with tc.tile_pool(name="lg_ps", bufs=2, space="PSUM") as lg_ps:
                for t in range(nT):
                    te = slice(t * E, (t + 1) * E)
                    ps = lg_ps.tile([P, E], f32, tag="lg", name="lg")
                    for k in range(nH):
                        nc.tensor.matmul(
                            ps,
                            xt[k][:, t * P : (t + 1) * P].bitcast(f32),
                            gate_sb[:, k * E : (k + 1) * E],
                            start=(k == 0),
                            stop=(k == nH - 1),
                        )
                    lt = rtmp_pool.tile([P, E], f32, tag="lt", name="lt")
                    nc.scalar.copy(lt, ps)
                    m1 = rtmp_pool.tile([P, 1], f32, tag="m1", name="m1")
                    nc.vector.reduce_max(m1, lt, axis=AX)
                    nm1 = rtmp_pool.tile([P, 1], f32, tag="nm1", name="nm1")
                    nc.vector.reduce_max(nm1, lt, axis=AX, negate=True)
                    nc.vector.tensor_scalar(mask1[:, te], lt, m1, None, op0=OP.is_ge)
                    lm = rtmp_pool.tile([P, E], f32, tag="lm", name="lm")
                    nc.vector.scalar_tensor_tensor(
                        lm, mask1[:, te], -1e30, lt, op0=OP.mult, op1=OP.add
                    )
                    m2 = rtmp_pool.tile([P, 1], f32, tag="m2", name="m2")
                    nc.vector.reduce_max(m2, lm, axis=AX)
                    nc.vector.tensor_scalar(mask2[:, te], lm, m2, None, op0=OP.is_ge)
                    d = rtmp_pool.tile([P, 1], f32, tag="d", name="d")
                    nc.vector.tensor_sub(d, m1, m2)
                    p1 = rtmp_pool.tile([P, 1], f32, tag="p1", name="p1")
                    nc.scalar.activation(p1, d, AF.Sigmoid)
                    p2 = rtmp_pool.tile([P, 1], f32, tag="p2", name="p2")
                    nc.scalar.activation(p2, d, AF.Sigmoid, scale=-1.0)
                    ca = rtmp_pool.tile([P, 1], f32, tag="ca", name="ca")
                    nc.vector.tensor_mul(ca, mask1[:, t * E : t * E + 1], p1)
                    cb = rtmp_pool.tile([P, 1], f32, tag="cb", name="cb")
                    nc.vector.tensor_mul(cb, mask2[:, t * E : t * E + 1], p2)
                    nc.vector.tensor_add(coef[:, t : t + 1], ca, cb)
                    # softmax over all 8 logits (aux loss)
                    sm = rtmp_pool.tile([P, E], f32, tag="sm", name="sm")
                    nc.scalar.activation(sm, lt, AF.Exp, bias=nm1)
                    ssum = rtmp_pool.tile([P, 1], f32, tag="ssum", name="ssum")
                    nc.vector.reduce_sum(ssum, sm, axis=AX)
                    rcp = rtmp_pool.tile([P, 1], f32, tag="rcp", name="rcp")
                    nc.vector.reciprocal(rcp, ssum)
                    nc.vector.tensor_scalar(smn[:, te], sm, rcp, None, op0=OP.mult)

            # ---- main expert FFN, one pass per token half; RS overlaps ----
            with (
                tc.tile_pool(name="ph1_ps", bufs=2, space="PSUM") as ph1_ps,
                tc.tile_pool(name="ph2_ps", bufs=3, space="PSUM") as ph2_ps,
            ):
                for hf in range(NHALF):
                    toff = hf * TH
                    acc = [
                        acc_pool.tile([P, H], f32, tag=f"acc{t}", name=f"acc{t}")
                        for t in range(nTh)
                    ]
                    for c in range(nFC):
                        ht = [
                            ht_pool.tile([P, TH], f32r, tag=f"ht{j}", name=f"ht{j}")
                            for j in range(fpc)
                        ]
                        # phase 1: hT[f, t] = silu(w1.T x) * (w3.T x)
                        for j in range(fpc):
                            ft = c * fpc + j
                            w1t, w3t = [], []
                            for k in range(nH):
                                a = wstream.tile([P, 2 * P], f32r, tag=f"w13_{k}", name=f"w13_{k}", bufs=3)
                                nc.sync.dma_start(a, w13_ext[ft, k].bitcast(f32r))
                                w1t.append(a[:, 0:P])
                                w3t.append(a[:, P : 2 * P])
                            for tb in range(nTBh):
                                xs = slice(toff + tb * TB, toff + (tb + 1) * TB)
                                ts_ = slice(tb * TB, (tb + 1) * TB)
                                g1 = ph1_ps.tile([P, TB], f32, tag="g1", name="g1")
                                g3 = ph1_ps.tile([P, TB], f32, tag="g3", name="g3")
                                for k in range(nH):
                                    nc.tensor.matmul(
                                        g1, w1t[k], xt[k][:, xs],
                                        start=(k == 0), stop=(k == nH - 1),
                                    )
                                for k in range(nH):
                                    nc.tensor.matmul(
                                        g3, w3t[k], xt[k][:, xs],
                                        start=(k == 0), stop=(k == nH - 1),
                                    )
                                s1 = silu_pool.tile([P, TB], f32, tag="s1", name="s1")
                                nc.scalar.activation(s1, g1, AF.Silu)
                                nc.vector.tensor_mul(ht[j][:, ts_], s1, g3)
                        # phase 2: acc[t, h] += hT.T @ w2
                        for hh in range(nHH):
                            hs = slice(hh * HH, (hh + 1) * HH)
                            w2t = []
                            for j in range(fpc):
                                ft = c * fpc + j
                                a = wstream.tile([P, HH], f32r, tag=f"w2_{j}", name=f"w2_{j}", bufs=3)
                                nc.sync.dma_start(a, w2_ext[ft, hh].bitcast(f32r))
                                w2t.append(a)
                            for t in range(nTh):
                                ps = ph2_ps.tile([P, HH], f32, tag="o", name="o")
                                for j in range(fpc):
                                    nc.tensor.matmul(
                                        ps, ht[j][:, t * P : (t + 1) * P], w2t[j],
                                        start=(j == 0), stop=(j == fpc - 1),
                                    )
                                if c == 0:
                                    nc.vector.tensor_copy(acc[t][:, hs], ps)
                                else:
                                    nc.vector.tensor_add(acc[t][:, hs], acc[t][:, hs], ps)

                    # scale by routing coef, write out, reduce-scatter this half
                    y_dram = dram_pool.tile([TH, H], f32, tag=f"y{hf}", name=f"y{hf}")
                    rs_dram = dram_pool.tile([P, H], f32, tag=f"rs{hf}", name=f"rs{hf}")
                    for t in range(nTh):
                        tg = hf * nTh + t
                        nc.scalar.activation(
                            acc[t], acc[t], AF.Copy, scale=coef[:, tg : tg + 1]
                        )
                        nc.sync.dma_start(y_dram[t * P : (t + 1) * P, :], acc[t])
                    nc.gpsimd.collective_compute(
                        "ReduceScatter",
                        OP.add,
                        replica_groups=[list(range(NCORES))],
                        ins=[y_dram.opt()],
                        outs=[rs_dram.opt()],
                    )
                    nc.gpsimd.dma_start(out_ext[hf * P : (hf + 1) * P, :], rs_dram)

            # ---- aux loss: E/T^2 * sum_e (sum_t m1 + sum_t m2)_e * (sum_t sm)_e ----
            with tc.tile_pool(name="aux_ps", bufs=1, space="PSUM") as aux_ps:
                aps = aux_ps.tile([1, 3 * E], f32)
                for i, src in enumerate((mask1, mask2, smn)):
                    for t in range(nT):
                        nc.tensor.matmul(
                            aps[:, i * E : (i + 1) * E],
                            ones,
                            src[:, t * E : (t + 1) * E],
                            start=(t == 0),
                            stop=(t == nT - 1),
                        )
                asb = rtmp_pool.tile([1, 3 * E], f32, tag="asb", name="asb")
                nc.scalar.copy(asb, aps)
                a1 = rtmp_pool.tile([1, E], f32, tag="a1", name="a1")
                nc.vector.tensor_add(a1, asb[:, 0:E], asb[:, E : 2 * E])
                a2 = rtmp_pool.tile([1, E], f32, tag="a2", name="a2")
                nc.vector.tensor_mul(a2, a1, asb[:, 2 * E : 3 * E])
                a3 = rtmp_pool.tile([1, 1], f32, tag="a3", name="a3")
                nc.vector.reduce_sum(a3, a2, axis=AX)
                aux_sb = rtmp_pool.tile([1, 1], f32, tag="aux_sb", name="aux_sb")
                nc.scalar.mul(aux_sb, a3, float(E) / float(T * T))
                nc.gpsimd.dma_start(aux_ext, aux_sb)

    nc.compile()
    return nc


def _get_nc():
    with _lock:
        if "nc" not in _compiled:
            _compiled["nc"] = _build()
        return _compiled["nc"]


# test.py can set TRACE=True to capture a neuron profile; the resulting
# BassKernelResults lands in LAST_RESULT.
TRACE = False
LAST_RESULT = None


def kernel(**inputs):
    global LAST_RESULT
    from concourse.bass_utils import run_bass_kernel_spmd

    hs = np.ascontiguousarray(
        np.asarray(inputs["hidden_states"], dtype=np.float32).reshape(T, H)
    )
    gate_w = np.asarray(inputs["gate_w"], dtype=np.float32)
    w1 = np.asarray(inputs["w1"], dtype=np.float32)
    w3 = np.asarray(inputs["w3"], dtype=np.float32)
    w2 = np.asarray(inputs["w2"], dtype=np.float32)

    x_ht = np.ascontiguousarray(hs[_token_perm()].T)  # [H, T]
    # weight tiles repacked contiguous: w1+w3 packed [nF, nH, P, 2P], w2 [nF, nHH, P, HH]
    w1_t = w1.reshape(NCORES, nH, P, nF, P).transpose(0, 3, 1, 2, 4)
    w3_t = w3.reshape(NCORES, nH, P, nF, P).transpose(0, 3, 1, 2, 4)
    w13_t = np.concatenate([w1_t, w3_t], axis=-1)  # [NCORES, nF, nH, P, 2P]
    w2_t = w2.reshape(NCORES, nF, P, nHH, HH).transpose(0, 1, 3, 2, 4)

    nc = _get_nc()
    in_maps = []
    for c in range(NCORES):
        in_maps.append(
            {
                "x": x_ht,
                "gate": np.ascontiguousarray(np.roll(gate_w, -c, axis=1)),
                "w13": np.ascontiguousarray(w13_t[c]),
                "w2": np.ascontiguousarray(w2_t[c]),
            }
        )

    res = run_bass_kernel_spmd(nc, in_maps, core_ids=list(range(NCORES)), trace=TRACE)
    LAST_RESULT = res

    final = np.concatenate(
        [res.results[c]["out"] for c in range(NCORES)], axis=0
    ).reshape(B, S, H)
    aux = np.asarray(res.results[0]["aux"], dtype=np.float32).reshape(())
    return final, aux


# revision 18
# speedup vs baseline: 1.0998x; 1.0186x over previous
"""ArcticMoE (dense top-2 MoE, 8 experts) — Trainium2 Bass kernel, 8 NeuronCores.

Sharding: expert-parallel. Core c receives expert c's weights (w1/w3/w2[c]),
the full token stream, and gate_w with columns rolled so core c's own expert
sits at column 0 (top-k selection and the aux loss are permutation-invariant).
Each core computes coef_c[t] * (silu(x@w1c) * (x@w3c)) @ w2c for all tokens;
ReduceScatter(add) sums over experts and scatters token shards; the host
concatenates the per-core shards.

Tokens are processed in two halves so the first half's ReduceScatter overlaps
the second half's compute. The host permutes token rows such that an 8-way
128-row scatter of each half lands exactly on sub-blocks of each core's final
256-row shard — no inverse permutation is needed on the output.

Compute dtype: float32r (full-rate PE streaming with fp32 storage) for the
big matmuls; the router matmuls run in plain fp32 so the top-2 selection
matches the fp32 reference. The aux load-balancing loss is computed on-device
via ones-vector matmul column sums of the top-1/top-2 masks and softmax probs.
"""

import threading

import numpy as np

B, S, H, F, E = 2, 1024, 1024, 4096, 8
T = B * S                    # 2048 tokens
P = 128                      # partitions
NCORES = 8
TSH = T // NCORES            # 256-row output shard per core

nH = H // P                  # 8  k-tiles over hidden dim
nT = T // P                  # 16 token tiles
nF = F // P                  # 32 f tiles
NHALF = 2                    # token passes (RS overlap granularity)
HSPLIT = (8, 8)              # t_tiles per pass (multiples of TB//P so N stays 512)
HOFF = (0, 8)                # t_tile offset per pass
FCH = 512                    # f-chunk width (phase-1 -> phase-2 granularity)
fpc = FCH // P               # 4  f-tiles per chunk
nFC = F // FCH               # 8 chunks
TB = 512                     # token block (matmul moving dim)
HH = 512                     # h_out half width (phase-2 psum free dim)
nHH = H // HH                # 2

_lock = threading.Lock()
_compiled = {}


def _token_perm():
    # pass h covers HSPLIT[h]*P tokens; an 8-way row scatter of each pass must
    # land on consecutive sub-blocks of every core's 256-row shard
    perm = np.empty(T, dtype=np.int64)
    idx = 0
    row_off = 0
    for h in range(NHALF):
        rows = HSPLIT[h] * P // NCORES
        for c in range(NCORES):
            base = c * TSH + row_off
            perm[idx : idx + rows] = np.arange(base, base + rows)
            idx += rows
        row_off += rows
    return perm


def _build():
    import concourse.mybir as mybir
    import concourse.tile as tile
    from concourse import bacc

    f32 = mybir.dt.float32
    f32r = mybir.dt.float32r
    AX = mybir.AxisListType.X
    AF = mybir.ActivationFunctionType
    OP = mybir.AluOpType

    nc = bacc.Bacc(
        "TRN2",
        target_bir_lowering=False,
        debug=False,
        enable_asserts=False,
        num_devices=NCORES,
    )

    x_ext = nc.dram_tensor("x", [H, T], f32, kind="ExternalInput").ap()
    gate_ext = nc.dram_tensor("gate", [H, E], f32, kind="ExternalInput").ap()
    w13_ext = nc.dram_tensor("w13", [nF, nH, P, 2 * P], f32, kind="ExternalInput").ap()
    w2_ext = nc.dram_tensor("w2", [nF, nHH, P, HH], f32, kind="ExternalInput").ap()
    out_ext = nc.dram_tensor("out", [TSH, H], f32, kind="ExternalOutput").ap()
    aux_ext = nc.dram_tensor("aux", [1, 1], f32, kind="ExternalOutput").ap()

    with tile.TileContext(nc) as tc:
        with (
            tc.tile_pool(name="const", bufs=1) as const_pool,
            tc.tile_pool(name="xt", bufs=1) as xt_pool,
            tc.tile_pool(name="acc", bufs=1) as acc_pool,
            tc.tile_pool(name="ht", bufs=1) as ht_pool,
            tc.tile_pool(name="wstream", bufs=2) as wstream,
            tc.tile_pool(name="silu", bufs=3) as silu_pool,
            tc.tile_pool(name="router", bufs=1) as router_pool,
            tc.tile_pool(name="rtmp", bufs=2) as rtmp_pool,
            tc.tile_pool(name="dram", bufs=1, space="DRAM") as dram_pool,
        ):
            ones = const_pool.tile([P, 1], f32)
            nc.vector.memset(ones, 1.0)
            gate_sb = const_pool.tile([P, nH * E], f32)
            for k in range(nH):
                nc.sync.dma_start(
                    gate_sb[:, k * E : (k + 1) * E],
                    gate_ext[k * P : (k + 1) * P, :],
                )

            xt = [xt_pool.tile([P, T], f32r, tag=f"xt{k}", name=f"xt{k}") for k in range(nH)]

            mask1 = router_pool.tile([P, nT * E], f32)
            mask2 = router_pool.tile([P, nT * E], f32)
            smn = router_pool.tile([P, nT * E], f32)
            coef = router_pool.tile([P, nT], f32)

            # ---- load xT directly (host supplies [H, T] layout) ----
            # pass-A token chunks first; pass-B chunks are emitted later so the
            # head DMAs only cover what the first matmuls need
            def load_xt_chunk(tc4):
                cs = slice(tc4 * (T // 4), (tc4 + 1) * (T // 4))
                for k in range(nH):
                    nc.sync.dma_start(
                        xt[k][:, cs], x_ext[k * P : (k + 1) * P, cs].bitcast(f32r)
                    )

            load_xt_chunk(0)
            load_xt_chunk(1)

            # ---- main expert FFN, one pass per token half; RS overlaps ----
            with (
                tc.tile_pool(name="ph1_ps", bufs=2, space="PSUM") as ph1_ps,
                tc.tile_pool(name="ph2_ps", bufs=3, space="PSUM") as ph2_ps,
            ):
                for hf in range(NHALF):
                    nTh_ = HSPLIT[hf]
                    toff = HOFF[hf] * P
                    TH_ = nTh_ * P
                    nTBh_ = TH_ // TB
                    acc = [
                        acc_pool.tile([P, H], f32, tag=f"acc{t}", name=f"acc{t}")
                        for t in range(nTh_)
                    ]
                    for c in range(nFC):
                        ht = [
                            ht_pool.tile([P, TH_], f32r, tag=f"ht{j}", name=f"ht{j}")
                            for j in range(fpc)
                        ]
                        # phase 1: hT[f, t] = silu(w1.T x) * (w3.T x)
                        for j in range(fpc):
                            ft = c * fpc + j
                            w1t, w3t = [], []
                            for k in range(nH):
                                a = wstream.tile([P, 2 * P], f32r, tag=f"w13_{k}", name=f"w13_{k}", bufs=3)
                                nc.sync.dma_start(a, w13_ext[ft, k].bitcast(f32r))
                                w1t.append(a[:, 0:P])
                                w3t.append(a[:, P : 2 * P])
                            for tb in range(nTBh_):
                                xs = slice(toff + tb * TB, toff + (tb + 1) * TB)
                                ts_ = slice(tb * TB, (tb + 1) * TB)
                                g1 = ph1_ps.tile([P, TB], f32, tag="g1", name="g1")
                                g3 = ph1_ps.tile([P, TB], f32, tag="g3", name="g3")
                                for k in range(nH):
                                    nc.tensor.matmul(
                                        g1, w1t[k], xt[k][:, xs],
                                        start=(k == 0), stop=(k == nH - 1),
                                    )
                                for k in range(nH):
                                    nc.tensor.matmul(
                                        g3, w3t[k], xt[k][:, xs],
                                        start=(k == 0), stop=(k == nH - 1),
                                    )
                                s1 = silu_pool.tile([P, TB], f32, tag="s1", name="s1")
                                nc.scalar.activation(s1, g1, AF.Silu)
                                nc.vector.tensor_mul(ht[j][:, ts_], s1, g3)
                        if hf == 0 and c == 0:
                            load_xt_chunk(2)
                            load_xt_chunk(3)
                        # phase 2: acc[t, h] += hT.T @ w2
                        for hh in range(nHH):
                            hs = slice(hh * HH, (hh + 1) * HH)
                            w2t = []
                            for j in range(fpc):
                                ft = c * fpc + j
                                a = wstream.tile([P, HH], f32r, tag=f"w2_{j}", name=f"w2_{j}", bufs=2)
                                nc.sync.dma_start(a, w2_ext[ft, hh].bitcast(f32r))
                                w2t.append(a)
                            for t in range(nTh_):
                                ps = ph2_ps.tile([P, HH], f32, tag="o", name="o")
                                for j in range(fpc):
                                    nc.tensor.matmul(
                                        ps, ht[j][:, t * P : (t + 1) * P], w2t[j],
                                        start=(j == 0), stop=(j == fpc - 1),
                                    )
                                if c == 0:
                                    nc.vector.tensor_copy(acc[t][:, hs], ps)
                                else:
                                    nc.vector.tensor_add(acc[t][:, hs], acc[t][:, hs], ps)

                    if hf == 0:
                        # ---- router: logits -> top-2 coefs + softmax stats (fp32 exact) ----
                        with tc.tile_pool(name="lg_ps", bufs=1, space="PSUM") as lg_ps:
                            for t in range(nT):
                                te = slice(t * E, (t + 1) * E)
                                ps = lg_ps.tile([P, E], f32, tag="lg", name="lg")
                                for k in range(nH):
                                    nc.tensor.matmul(
                                        ps,
                                        xt[k][:, t * P : (t + 1) * P].bitcast(f32),
                                        gate_sb[:, k * E : (k + 1) * E],
                                        start=(k == 0),
                                        stop=(k == nH - 1),
                                    )
                                lt = rtmp_pool.tile([P, E], f32, tag="lt", name="lt")
                                nc.scalar.copy(lt, ps)
                                m1 = rtmp_pool.tile([P, 1], f32, tag="m1", name="m1")
                                nc.vector.reduce_max(m1, lt, axis=AX)
                                nm1 = rtmp_pool.tile([P, 1], f32, tag="nm1", name="nm1")
                                nc.vector.reduce_max(nm1, lt, axis=AX, negate=True)
                                nc.vector.tensor_scalar(mask1[:, te], lt, m1, None, op0=OP.is_ge)
                                lm = rtmp_pool.tile([P, E], f32, tag="lm", name="lm")
                                nc.vector.scalar_tensor_tensor(
                                    lm, mask1[:, te], -1e30, lt, op0=OP.mult, op1=OP.add
                                )
                                m2 = rtmp_pool.tile([P, 1], f32, tag="m2", name="m2")
                                nc.vector.reduce_max(m2, lm, axis=AX)
                                nc.vector.tensor_scalar(mask2[:, te], lm, m2, None, op0=OP.is_ge)
                                d = rtmp_pool.tile([P, 1], f32, tag="d", name="d")
                                nc.vector.tensor_sub(d, m1, m2)
                                p1 = rtmp_pool.tile([P, 1], f32, tag="p1", name="p1")
                                nc.scalar.activation(p1, d, AF.Sigmoid)
                                p2 = rtmp_pool.tile([P, 1], f32, tag="p2", name="p2")
                                nc.scalar.activation(p2, d, AF.Sigmoid, scale=-1.0)
                                ca = rtmp_pool.tile([P, 1], f32, tag="ca", name="ca")
                                nc.vector.tensor_mul(ca, mask1[:, t * E : t * E + 1], p1)
                                cb = rtmp_pool.tile([P, 1], f32, tag="cb", name="cb")
                                nc.vector.tensor_mul(cb, mask2[:, t * E : t * E + 1], p2)
                                nc.vector.tensor_add(coef[:, t : t + 1], ca, cb)
                                # softmax over all 8 logits (aux loss)
                                sm = rtmp_pool.tile([P, E], f32, tag="sm", name="sm")
                                nc.scalar.activation(sm, lt, AF.Exp, bias=nm1)
                                ssum = rtmp_pool.tile([P, 1], f32, tag="ssum", name="ssum")
                                nc.vector.reduce_sum(ssum, sm, axis=AX)
                                rcp = rtmp_pool.tile([P, 1], f32, tag="rcp", name="rcp")
                                nc.vector.reciprocal(rcp, ssum)
                                nc.vector.tensor_scalar(smn[:, te], sm, rcp, None, op0=OP.mult)

                    # scale by routing coef, write out, reduce-scatter this pass
                    rsrows = TH_ // NCORES
                    y_dram = dram_pool.tile([TH_, H], f32, tag=f"y{hf}", name=f"y{hf}")
                    rs_dram = dram_pool.tile([rsrows, H], f32, tag=f"rs{hf}", name=f"rs{hf}")
                    for t in range(nTh_):
                        tg = HOFF[hf] + t
                        nc.scalar.activation(
                            acc[t], acc[t], AF.Copy, scale=coef[:, tg : tg + 1]
                        )
                        nc.sync.dma_start(y_dram[t * P : (t + 1) * P, :], acc[t])
                    nc.gpsimd.collective_compute(
                        "ReduceScatter",
                        OP.add,
                        replica_groups=[list(range(NCORES))],
                        ins=[y_dram.opt()],
                        outs=[rs_dram.opt()],
                    )
                    nc.gpsimd.dma_start(
                        out_ext[HOFF[hf] * P // NCORES : (HOFF[hf] + nTh_) * P // NCORES, :],
                        rs_dram,
                    )

            # ---- aux loss: E/T^2 * sum_e (sum_t m1 + sum_t m2)_e * (sum_t sm)_e ----
            with tc.tile_pool(name="aux_ps", bufs=1, space="PSUM") as aux_ps:
                aps = aux_ps.tile([1, 3 * E], f32)
                for i, src in enumerate((mask1, mask2, smn)):
                    for t in range(nT):
                        nc.tensor.matmul(
                            aps[:, i * E : (i + 1) * E],
                            ones,
                            src[:, t * E : (t + 1) * E],
                            start=(t == 0),
                            stop=(t == nT - 1),
                        )
                asb = rtmp_pool.tile([1, 3 * E], f32, tag="asb", name="asb")
                nc.scalar.copy(asb, aps)
                a1 = rtmp_pool.tile([1, E], f32, tag="a1", name="a1")
                nc.vector.tensor_add(a1, asb[:, 0:E], asb[:, E : 2 * E])
                a2 = rtmp_pool.tile([1, E], f32, tag="a2", name="a2")
                nc.vector.tensor_mul(a2, a1, asb[:, 2 * E : 3 * E])
                a3 = rtmp_pool.tile([1, 1], f32, tag="a3", name="a3")
                nc.vector.reduce_sum(a3, a2, axis=AX)
                aux_sb = rtmp_pool.tile([1, 1], f32, tag="aux_sb", name="aux_sb")
                nc.scalar.mul(aux_sb, a3, float(E) / float(T * T))
                nc.gpsimd.dma_start(aux_ext, aux_sb)

    nc.compile()
    return nc


def _get_nc():
    with _lock:
        if "nc" not in _compiled:
            _compiled["nc"] = _build()
        return _compiled["nc"]


# test.py can set TRACE=True to capture a neuron profile; the resulting
# BassKernelResults lands in LAST_RESULT.
TRACE = False
LAST_RESULT = None


def kernel(**inputs):
    global LAST_RESULT
    from concourse.bass_utils import run_bass_kernel_spmd

    hs = np.ascontiguousarray(
        np.asarray(inputs["hidden_states"], dtype=np.float32).reshape(T, H)
    )
    gate_w = np.asarray(inputs["gate_w"], dtype=np.float32)
    w1 = np.asarray(inputs["w1"], dtype=np.float32)
    w3 = np.asarray(inputs["w3"], dtype=np.float32)
    w2 = np.asarray(inputs["w2"], dtype=np.float32)

    x_ht = np.ascontiguousarray(hs[_token_perm()].T)  # [H, T]
    # weight tiles repacked contiguous: w1+w3 packed [nF, nH, P, 2P], w2 [nF, nHH, P, HH]
    w1_t = w1.reshape(NCORES, nH, P, nF, P).transpose(0, 3, 1, 2, 4)
    w3_t = w3.reshape(NCORES, nH, P, nF, P).transpose(0, 3, 1, 2, 4)
    w13_t = np.concatenate([w1_t, w3_t], axis=-1)  # [NCORES, nF, nH, P, 2P]
    w2_t = w2.reshape(NCORES, nF, P, nHH, HH).transpose(0, 1, 3, 2, 4)

    nc = _get_nc()
    in_maps = []
    for c in range(NCORES):
        in_maps.append(
            {
                "x": x_ht,
                "gate": np.ascontiguousarray(np.roll(gate_w, -c, axis=1)),
                "w13": np.ascontiguousarray(w13_t[c]),
                "w2": np.ascontiguousarray(w2_t[c]),
            }
        )

    res = run_bass_kernel_spmd(nc, in_maps, core_ids=list(range(NCORES)), trace=TRACE)
    LAST_RESULT = res

    final = np.concatenate(
        [res.results[c]["out"] for c in range(NCORES)], axis=0
    ).reshape(B, S, H)
    aux = np.asarray(res.results[0]["aux"], dtype=np.float32).reshape(())
    return final, aux

